# revision 1
# baseline (speedup 1.0000x reference)
"""Trainium2 Bass kernel for 4-directional cumulative-max corner pooling.

reference: p = x[:, :16]; out = concat([x, cummax_H(p), cummax_H_rev(p),
                                        cummax_W(p), cummax_W_rev(p)], axis=1)
x: [32, 64, 128, 128] f32 -> out: [32, 128, 128, 128] f32

Sharding: data-parallel over batch, 4 batches per core on 8 cores; no
cross-core communication. Per-core plan (b = 0..3):
  - load x[b, :16] as one SBUF tile [H=128 part, (c w) free]  (1 MiB DMA)
  - W-direction cummax via the native DVE prefix-scan (TensorTensorScanArith),
    reverse direction via negative-stride access patterns
  - H-direction cummax by PE-transposing each 128x128 slice into PSUM,
    scanning along the (now-free) H axis, and PE-transposing back; the
    PSUM->SBUF copies are split ACT/DVE to balance engine busy time
  - passthrough: out[b, :16] is re-stored from the already-loaded input tile
    (input is read exactly once); out[b, 16:64] alternates between a direct
    HBM->HBM DMA on the Pool/SWDGE ring and an SBUF bounce split across the
    two HWDGE rings
  - every DMA ring serializes its own transfers, so loads/stores/passthrough
    are spread over the SP ring, the ACT ring and the SWDGE ring such that
    no ring (and no engine) exceeds ~70 us of work per shard

Measured (rep-slope wall clock, 8 cores concurrent): ~90-102 us per core
depending on session (best measured 89.6 us); a stores/loads-only variant of
the same byte pattern measures ~95 us in the same conditions, i.e. the
kernel sits at the DMA/HBM floor with compute fully hidden.
"""

import numpy as np
from contextlib import ExitStack

import concourse.bass as bass
import concourse.bacc as bacc
import concourse.mybir as mybir
from concourse import masks
from concourse.tile import TileContext
from concourse.bass_utils import run_bass_kernel_spmd

B_TOTAL, C_IN, H, W = 32, 64, 128, 128
PICK = 16
N_CORES = 8
B_PER = B_TOTAL // N_CORES
C_OUT = C_IN + 4 * PICK
F32 = mybir.dt.float32
NEG = -3.4028234663852886e38  # finite f32 lowest; identity for max on randn data


def _emit(
    ctx: ExitStack, tc: TileContext, x: bass.AP, out: bass.AP, reps: int = 1
) -> None:
    nc = tc.nc
    MAX = mybir.AluOpType.max

    const_pool = ctx.enter_context(tc.tile_pool(name="const", bufs=1))
    ident = const_pool.tile([128, 128], F32)
    masks.make_identity(nc, ident[:])
    neginf = const_pool.tile([128, 128], F32)
    nc.gpsimd.memset(neginf[:], NEG)

    in_pool = ctx.enter_context(tc.tile_pool(name="tin", bufs=2))
    pb_pool = ctx.enter_context(tc.tile_pool(name="pb", bufs=2))
    out_pool = ctx.enter_context(tc.tile_pool(name="tout", bufs=2))
    small_pool = ctx.enter_context(tc.tile_pool(name="small", bufs=3))
    psum_a = ctx.enter_context(
        tc.tile_pool(name="psa", bufs=3, space=bass.MemorySpace.PSUM)
    )
    psum_b = ctx.enter_context(
        tc.tile_pool(name="psb", bufs=2, space=bass.MemorySpace.PSUM)
    )

    def scan(dst: bass.AP, src: bass.AP, reverse: bool) -> None:
        # cummax along the free dim: state = max(src[:, t], state); data1 is
        # the -inf constant tile so op1=max is an identity.
        if reverse:
            dst, src = dst[:, ::-1], src[:, ::-1]
        nc.vector.tensor_tensor_scan(dst, src, neginf[:], NEG, MAX, MAX)

    for _rep in range(reps):
      for b in range(B_PER):
        tin = in_pool.tile([128, PICK * W], F32)
        tin3 = tin[:].rearrange("h (c w) -> h c w", w=W)
        nc.sync.dma_start(out=tin3, in_=x[b, 0:PICK].rearrange("c h w -> h c w"))

        # Passthrough. Each DMA ring serializes its own transfers
        # (~3.2 us/MiB strided, ~2.5 us/MiB contiguous; DRAM->DRAM ~25 us
        # per 3 MiB charged to the issuing engine), so the byte budget is
        # spread across sync(SP), ACT and the Pool/SWDGE ring.
        nc.scalar.dma_start(
            out=out[b, 0:PICK].rearrange("c h w -> h c w"), in_=tin3
        )
        if b % 2 == 0:
            # direct HBM->HBM on the otherwise-idle Pool ring
            nc.gpsimd.dma_start(out=out[b, PICK:C_IN], in_=x[b, PICK:C_IN])
        else:
            # SBUF bounce split across the two HWDGE rings
            pb = pb_pool.tile([128, (C_IN - PICK) * H * W // 128], F32)
            src_flat = x[b, PICK:C_IN].flatten().rearrange("(p f) -> p f", p=128)
            dst_flat = out[b, PICK:C_IN].flatten().rearrange("(p f) -> p f", p=128)
            nc.sync.dma_start(out=pb[:], in_=src_flat)
            nc.scalar.dma_start(out=dst_flat, in_=pb[:])

        # down gets its own tile; up+right+left share one tile so output
        # channels 80:128 ship as a single 3 MiB DMA (all three blocks are
        # DVE-produced, so one store dependency; fewer per-DMA fixed costs)
        t_down = out_pool.tile([128, PICK * W], F32, tag="t_down")
        t_url = out_pool.tile([128, 3 * PICK * W], F32, tag="t_url")
        t_up = t_url[:, : PICK * W]
        t_right = t_url[:, PICK * W : 2 * PICK * W]
        t_left = t_url[:, 2 * PICK * W :]
        for c in range(PICK):
            src = tin[:, c * W : (c + 1) * W]
            cs = slice(c * W, (c + 1) * W)
            scan(t_right[:, cs], src, False)
            scan(t_left[:, cs], src, True)

            pt = psum_a.tile([128, 128], F32)
            nc.tensor.transpose(pt[:], src, ident[:])
            dt = small_pool.tile([128, 128], F32, tag="dt")
            ut = small_pool.tile([128, 128], F32, tag="ut")
            scan(dt[:], pt[:], False)
            scan(ut[:], pt[:], True)
            # transpose-backs aim at a shared full-bank PSUM tile (4 channels
            # per [128,512] bank) so the PSUM->SBUF copies batch 4 channels
            # per instruction - 4x fewer copies on ACT (down) and DVE (up);
            # measured -35 us on HW vs per-channel copies
            if c % 4 == 0:
                pd4 = psum_b.tile([128, 512], F32, tag="pd")
                pu4 = psum_b.tile([128, 512], F32, tag="pu")
            q = (c % 4) * 128
            nc.tensor.transpose(pd4[:, q : q + 128], dt[:], ident[:])
            nc.tensor.transpose(pu4[:, q : q + 128], ut[:], ident[:])
            if c % 4 == 3:
                nc.scalar.copy(t_down[:, (c - 3) * W : (c + 1) * W], pd4[:])
                nc.vector.tensor_copy(t_up[:, (c - 3) * W : (c + 1) * W], pu4[:])

        # down stores issue from ACT right after ACT's own down-copies
        # (waits already satisfied in program order); the merged up/right/
        # left store goes on the sync ring, whose only other work is loads.
        nc.scalar.dma_start(
            out=out[b, C_IN : C_IN + PICK].rearrange("c h w -> h c w"),
            in_=t_down[:].rearrange("h (c w) -> h c w", w=W),
        )
        nc.sync.dma_start(
            out=out[b, C_IN + PICK : C_IN + 4 * PICK].rearrange(
                "c h w -> h c w"
            ),
            in_=t_url[:].rearrange("h (c w) -> h c w", w=W),
        )


def build_nc(reps: int = 1) -> bass.Bass:
    # Bacc (not raw Bass): its compile() legalizes sync waits for TRN2
    # (max one wait per instruction; extra matmul waits move to ldweights).
    nc = bacc.Bacc("TRN2", target_bir_lowering=False, debug=False)
    x = nc.declare_dram_parameter("x", [B_PER, C_IN, H, W], F32, isOutput=False)
    out = nc.declare_dram_parameter("out", [B_PER, C_OUT, H, W], F32, isOutput=True)
    with TileContext(nc) as tc:
        with ExitStack() as ctx:
            _emit(ctx, tc, x, out, reps=reps)
    nc.compile()
    return nc


def kernel(x: np.ndarray, **_unused) -> np.ndarray:
    assert x.shape == (B_TOTAL, C_IN, H, W), x.shape
    nc = build_nc()
    in_maps = [
        {"x": np.ascontiguousarray(x[k * B_PER : (k + 1) * B_PER])}
        for k in range(N_CORES)
    ]
    res = run_bass_kernel_spmd(nc, in_maps, list(range(N_CORES)))
    return np.concatenate(
        [r["out"] for r in res.results], axis=0
    ).astype(np.float32)

